# revision 11
# baseline (speedup 1.0000x reference)
"""Causal self-attention Trainium2 kernel (B=2, T=2048, C=1024, H=16).

Sharding: 8 cores = 2 batches x 4 head-groups (4 heads/core, Megatron-style
column-parallel QKV + row-parallel proj; the row-parallel all-reduce is the
host-side partial sum in `kernel`).

Per-core schedule (bf16 matmul operands, fp32 PSUM accumulation), software-
pipelined so the in-order Tensor queue never drains:

  stage 0:  QKV for t-supertile 0
  stage s:  QKV for supertile s  zippered with  attention for supertile s-1
  (attention for supertile 3 is folded into stage 3's zipper)

Attention details:
  - qT/kT kept transposed [head_dim, T], 2 heads packed per 128 partitions;
    scores are computed transposed (sT[k, q] = k @ qT) with the two heads
    row-packed on the PE array via tile_position, both written into ONE
    [128, 1024] PSUM tile per k-block so the pair issues back-to-back and
    streams concurrently through separate PE row-quadrants.
  - one exp per k-block on ScalarE over the full [128, 1024] tile (dead
    columns hold stale-but-finite PSUM values and are never read downstream);
    causality = column-range restriction + one triangular-mask multiply per
    diagonal 128-block (on GpSimd, which is otherwise idle).
  - v is kept natural [T, 64] per head with a ones column appended, so the
    AV matmul emits softmax denominators as row 64 of the accumulator.
  - normalization: reciprocal_approx_fast on the denominator row, broadcast
    across partitions with gpsimd.partition_broadcast, then one multiply.
    (The baseline's fp32 rank-1 broadcast matmuls + bit-exact reciprocal
    cost 31us of PE + 53us of DVE; this costs ~1us of each.)
  - proj consumes yT directly; per-core output is a partial [T, C] product
    summed on the host (row-parallel all-reduce).
"""

import sys

for _p in ("/opt/trn_rl_repo",):
    if _p not in sys.path:
        sys.path.insert(0, _p)

import ml_dtypes
import numpy as np

import concourse.bacc as bacc
import concourse.mybir as mybir
import concourse.tile as tile
from concourse.alu_op_type import AluOpType
from concourse.bass_utils import run_bass_kernel_spmd

F32 = mybir.dt.float32
BF16 = mybir.dt.bfloat16
NPBF = ml_dtypes.bfloat16
EXP = mybir.ActivationFunctionType.Exp

B, T, C = 2, 2048, 1024
H, HD = 16, 64
HPC = 4          # heads per core
NPAIR = 2        # head pairs per core
CL = HPC * HD    # 256 local channels
NCORES = 8
SCALE = 0.125    # 1/sqrt(64), folded into wq host-side

TT5 = T // 512   # 4  t supertiles / q supertiles / pipeline stages
TT1 = T // 128   # 16 t tiles / k blocks
CCH = C // 128   # 8  contraction chunks


def _build_program():
    nc = bacc.Bacc("TRN2", target_bir_lowering=False, debug=False)

    xT_d = nc.dram_tensor("xT", [C, T], BF16, kind="ExternalInput").ap()
    wq_d = nc.dram_tensor("wq", [C, CL], BF16, kind="ExternalInput").ap()
    wk_d = nc.dram_tensor("wk", [C, CL], BF16, kind="ExternalInput").ap()
    wv_d = nc.dram_tensor("wv", [C, CL], BF16, kind="ExternalInput").ap()
    wp_d = nc.dram_tensor("wp", [CL, C], BF16, kind="ExternalInput").ap()
    bqs_d = nc.dram_tensor("bqs", [128, NPAIR], F32, kind="ExternalInput").ap()
    bks_d = nc.dram_tensor("bks", [128, NPAIR], F32, kind="ExternalInput").ap()
    bvr_d = nc.dram_tensor("bvr", [1, 2 * CL], BF16, kind="ExternalInput").ap()
    ones1_d = nc.dram_tensor("ones1", [1, 128], BF16, kind="ExternalInput").ap()
    mtri_d = nc.dram_tensor("mtri", [128, 128], BF16, kind="ExternalInput").ap()
    yp_d = nc.dram_tensor("yp", [T, C], F32, kind="ExternalOutput").ap()

    with tile.TileContext(nc) as tc:
        _attn_kernel(tc, xT_d, wq_d, wk_d, wv_d, wp_d, bqs_d, bks_d, bvr_d,
                     ones1_d, mtri_d, yp_d)
    nc.compile()
    return nc


def _attn_kernel(tc, xT_d, wq_d, wk_d, wv_d, wp_d, bqs_d, bks_d, bvr_d,
                 ones1_d, mtri_d, yp_d):
    nc = tc.nc
    mm = nc.tensor.matmul

    with (
        tc.tile_pool(name="const", bufs=1) as cpool,
        tc.tile_pool(name="big", bufs=1) as bigpool,
        tc.tile_pool(name="work", bufs=3) as wkpool,
        tc.tile_pool(name="outp", bufs=3) as opool,
        tc.tile_pool(name="spool", bufs=2, space="PSUM") as spool,
        tc.tile_pool(name="ytpool", bufs=1, space="PSUM") as ytpool,
        tc.tile_pool(name="rot", bufs=2, space="PSUM") as rotpool,
    ):
        # ---- constants ----
        bqs = cpool.tile([128, NPAIR], F32)
        nc.sync.dma_start(bqs, bqs_d)
        bks = cpool.tile([128, NPAIR], F32)
        nc.sync.dma_start(bks, bks_d)
        bvr2 = cpool.tile([1, 2 * CL], BF16)
        nc.sync.dma_start(bvr2, bvr_d)
        ones1 = cpool.tile([1, 128], BF16)
        nc.sync.dma_start(ones1, ones1_d)
        mtri = cpool.tile([128, 128], BF16)
        nc.sync.dma_start(mtri, mtri_d)

        # ---- SBUF residents ----
        xt = bigpool.tile([128, CCH, T], BF16)          # x^T chunks
        wqt = bigpool.tile([128, CCH, CL], BF16)
        wkt = bigpool.tile([128, CCH, CL], BF16)
        wvt = bigpool.tile([128, CCH, CL], BF16)
        wpt = bigpool.tile([128, NPAIR, C], BF16)       # proj weight chunks
        qt = bigpool.tile([128, NPAIR, T], BF16)        # q^T (scaled+biased)
        kt = bigpool.tile([128, NPAIR, T], BF16)        # k^T (biased)
        vt = bigpool.tile([128, TT1, HPC, HD + 1], BF16)  # v natural + ones
        yt = bigpool.tile([128, NPAIR, T], BF16)        # attn out ^T (normed)

        # input DMA: supertile-0 slices of x interleaved with the qkv
        # weights (everything stage 0 touches lands first), then the rest.
        for c in range(CCH):
            nc.sync.dma_start(xt[:, c, 0:512], xT_d[c * 128:(c + 1) * 128, 0:512])
            for w_sb, w_dr in ((wqt, wq_d), (wkt, wk_d), (wvt, wv_d)):
                nc.sync.dma_start(w_sb[:, c, :], w_dr[c * 128:(c + 1) * 128, :])
        for st in range(1, TT5):
            for c in range(CCH):
                nc.sync.dma_start(xt[:, c, st * 512:(st + 1) * 512],
                                  xT_d[c * 128:(c + 1) * 128,
                                       st * 512:(st + 1) * 512])
            if st == 1:
                for p in range(NPAIR):
                    nc.sync.dma_start(wpt[:, p, :],
                                      wp_d[p * 128:(p + 1) * 128, :])

        for tt in range(TT1):
            nc.vector.memset(vt[:, tt, :, HD:HD + 1], 1.0)

        # ================= step builders =================

        def qk_tile_step(st, p, w_sb, dst, bias):
            def run():
                pst = rotpool.tile([128, 512], F32, tag="rot",
                                   name=f"pqk_{st}_{p}_{id(w_sb)}")
                for c in range(CCH):
                    mm(pst,
                       w_sb[:, c, p * 128:(p + 1) * 128],
                       xt[:, c, st * 512:(st + 1) * 512],
                       start=(c == 0), stop=(c == CCH - 1))
                nc.vector.tensor_scalar_add(
                    dst[:, p, st * 512:(st + 1) * 512], pst, bias[:, p:p + 1])
            return run

        def v_tile_step(st, half):
            tt0 = 4 * st + 2 * half
            def run():
                psv = rotpool.tile([128, 512], F32, tag="rot",
                                   name=f"pv_{tt0}")
                # bias first: ONE start=True covering the tile (PSUM start
                # resets the whole 2KB bank, so per-half starts would clobber
                # each other's accumulation)
                mm(psv, ones1, bvr2, start=True, stop=False)
                for c in range(CCH):
                    for j in range(2):
                        mm(psv[:, 256 * j:256 * (j + 1)],
                           xt[:, c, (tt0 + j) * 128:(tt0 + j + 1) * 128],
                           wvt[:, c, :],
                           start=False,
                           stop=(c == CCH - 1 and j == 1))
                for j in range(2):
                    nc.vector.tensor_copy(
                        vt[:, tt0 + j, 0:HPC, 0:HD],
                        psv[:, 256 * j:256 * (j + 1)])
            return run

        def qkv_steps(st):
            steps = []
            for p in range(NPAIR):
                steps.append(qk_tile_step(st, p, wqt, qt, bqs))
                steps.append(qk_tile_step(st, p, wkt, kt, bks))
            steps.append(v_tile_step(st, 0))
            steps.append(v_tile_step(st, 1))
            return steps

        def att_steps(qst):
            q0 = qst * 512
            nkb = 4 * qst + 4
            steps = []
            ytps = {}
            ex_tiles = {}

            def score_step(p, kb):
                j = kb - 4 * qst
                vlo = 128 * j if j >= 0 else 0
                def run():
                    stile = spool.tile([128, 1024], F32, tag="s",
                                       name=f"s_{qst}_{p}_{kb}")
                    for hs in range(2):
                        r = slice(64 * hs, 64 * hs + 64)
                        mm(stile[:, 512 * hs + vlo:512 * (hs + 1)],
                           kt[r, p, kb * 128:(kb + 1) * 128],
                           qt[r, p, q0 + vlo:q0 + 512],
                           tile_position=(64 * hs, 0),
                           start=True, stop=True)
                    ex = wkpool.tile([128, 1024], BF16, tag="ex",
                                     name=f"ex_{qst}_{p}_{kb}", bufs=4)
                    nc.scalar.activation(ex, stile, EXP)
                    if j >= 0:
                        for hs in range(2):
                            band = slice(512 * hs + vlo, 512 * hs + vlo + 128)
                            nc.gpsimd.tensor_mul(ex[:, band], ex[:, band], mtri)
                    ex_tiles[(p, kb)] = ex
                return run

            def av_step(p, kb):
                j = kb - 4 * qst
                vlo = 128 * j if j >= 0 else 0
                def run():
                    ex = ex_tiles[(p, kb)]
                    for hs in range(2):
                        mm(ytps[p][:, 512 * hs + vlo:512 * (hs + 1)],
                           vt[:, kb, 2 * p + hs, :],
                           ex[:, 512 * hs + vlo:512 * (hs + 1)],
                           start=(kb == 0), stop=(kb == nkb - 1))
                return run

            def alloc_step(p):
                def run():
                    # both hs halves in one tile, split at the 512-column
                    # (= PSUM bank) boundary so each hs's start=True resets
                    # only its own bank
                    ytps[p] = ytpool.tile([HD + 1, 1024], F32, tag="yt",
                                          name=f"ytps_{qst}_{p}")
                return run

            def norm_step(p):
                def run():
                    # standard copy first: it carries the dependency on the
                    # AV matmuls (the custom-DVE reciprocal read the PSUM
                    # accumulator before AV finished when reading directly),
                    # and the in-order DVE queue then orders the custom op.
                    dsum = wkpool.tile([1, 1024], F32, tag="dsum",
                                       name=f"dsum_{qst}_{p}")
                    nc.vector.tensor_copy(dsum, ytps[p][HD:HD + 1, :])
                    sinv = wkpool.tile([1, 1024], F32, tag="sinv",
                                       name=f"sinv_{qst}_{p}")
                    nc.vector.reciprocal_approx_fast(sinv, dsum)
                    rbs = wkpool.tile([128, 1024], F32, tag="rbs",
                                      name=f"rbs_{qst}_{p}")
                    nc.gpsimd.partition_broadcast(rbs, sinv)
                    for hs in range(2):
                        nc.vector.tensor_mul(
                            yt[64 * hs:64 * hs + 64, p, q0:q0 + 512],
                            ytps[p][0:HD, 512 * hs:512 * (hs + 1)],
                            rbs[0:HD, 512 * hs:512 * (hs + 1)])
                return run

            def proj_step(tt, nh):
                def run():
                    pso = rotpool.tile([128, 512], F32, tag="rot",
                                       name=f"pso_{tt}_{nh}")
                    for p in range(NPAIR):
                        mm(pso,
                           yt[:, p, tt * 128:(tt + 1) * 128],
                           wpt[:, p, nh * 512:(nh + 1) * 512],
                           start=(p == 0), stop=(p == NPAIR - 1))
                    osb = opool.tile([128, 512], F32, tag="osb",
                                     name=f"osb_{tt}_{nh}")
                    nc.vector.tensor_copy(osb, pso)
                    nc.sync.dma_start(
                        yp_d[tt * 128:(tt + 1) * 128,
                             nh * 512:(nh + 1) * 512], osb)
                return run

            for p in range(NPAIR):
                steps.append(alloc_step(p))
                # AV lags scores by 2 k-blocks so the exp (ScalarE) and its
                # semaphore hops hide behind two score+filler slots
                for kb in range(nkb):
                    steps.append(score_step(p, kb))
                    if kb >= 2:
                        steps.append(av_step(p, kb - 2))
                steps.append(av_step(p, nkb - 2))
                steps.append(av_step(p, nkb - 1))
                steps.append(norm_step(p))
            for tt in range(4 * qst, 4 * qst + 4):
                for nh in range(2):
                    steps.append(proj_step(tt, nh))
            return steps

        def zipper(a, b, b_end=None):
            """Interleave two step lists; b is paced so it finishes by
            a-index b_end (default: end of a)."""
            na, nb = len(a), len(b)
            if b_end is None:
                b_end = na
            ia = ib = 0
            while ia < na or ib < nb:
                if ib >= nb or (ia < na and ia * nb <= ib * b_end):
                    a[ia]()
                    ia += 1
                else:
                    b[ib]()
                    ib += 1

        # ================= pipeline =================
        for s in qkv_steps(0):
            s()
        zipper(att_steps(0), qkv_steps(1))
        zipper(att_steps(1), qkv_steps(2))
        a2, a3 = att_steps(2), att_steps(3)
        # qkv(3) must be fully emitted before att(3) reads its q/k/v
        zipper(a2 + a3, qkv_steps(3), b_end=len(a2))


_CACHE = {}


def _get_nc():
    if "nc" not in _CACHE:
        _CACHE["nc"] = _build_program()
    return _CACHE["nc"]


def make_in_maps(x, w_attn, b_attn):
    """Shard the full inputs into 8 per-core input maps."""
    x = np.asarray(x, dtype=np.float32)
    w_attn = np.asarray(w_attn, dtype=np.float32)
    b_attn = np.asarray(b_attn, dtype=np.float32)
    ones1 = np.ones((1, 128), dtype=NPBF)
    mtri = (np.arange(128)[None, :] >= np.arange(128)[:, None]).astype(NPBF)
    in_maps = []
    for core in range(NCORES):
        b, g = divmod(core, 4)
        cs = slice(g * CL, (g + 1) * CL)
        ks = slice(C + g * CL, C + (g + 1) * CL)
        vs = slice(2 * C + g * CL, 2 * C + (g + 1) * CL)
        bv = b_attn[vs].reshape(1, CL).astype(NPBF)
        in_maps.append({
            "xT": np.ascontiguousarray(x[b].T).astype(NPBF),
            "wq": np.ascontiguousarray(SCALE * w_attn[:, cs]).astype(NPBF),
            "wk": np.ascontiguousarray(w_attn[:, ks]).astype(NPBF),
            "wv": np.ascontiguousarray(w_attn[:, vs]).astype(NPBF),
            "wp": None,  # filled by caller (needs w_proj)
            "bqs": np.ascontiguousarray(
                (SCALE * b_attn[cs]).reshape(NPAIR, 128).T),
            "bks": np.ascontiguousarray(b_attn[ks].reshape(NPAIR, 128).T),
            "bvr": np.concatenate([bv, bv], axis=1),
            "ones1": ones1,
            "mtri": mtri,
        })
    return in_maps


def kernel(x, w_attn, b_attn, w_proj, b_proj, _trace=False):
    w_proj = np.asarray(w_proj, dtype=np.float32)
    b_proj = np.asarray(b_proj, dtype=np.float32)
    in_maps = make_in_maps(x, w_attn, b_attn)
    for core in range(NCORES):
        g = core % 4
        in_maps[core]["wp"] = np.ascontiguousarray(
            w_proj[g * CL:(g + 1) * CL, :]).astype(NPBF)
    nc = _get_nc()
    res = run_bass_kernel_spmd(nc, in_maps, core_ids=list(range(NCORES)),
                               trace=_trace)
    out = np.zeros((B, T, C), dtype=np.float32)
    for core in range(NCORES):
        out[core // 4] += res.results[core]["yp"]
    out += b_proj
    if _trace:
        kernel.last_result = res
    return out


# revision 17
# speedup vs baseline: 1.3274x; 1.3274x over previous
"""Causal self-attention Trainium2 kernel (B=2, T=2048, C=1024, H=16).

Sharding: 8 cores = 2 batches x 4 head-groups (4 heads/core, Megatron-style
column-parallel QKV + row-parallel proj; the row-parallel all-reduce is the
host-side partial sum in `kernel`).

Per-core schedule (bf16 matmul operands, fp32 PSUM accumulation), software-
pipelined so the in-order Tensor queue never drains:

  stage 0:  QKV for t-supertile 0
  stage s:  QKV for supertile s  zippered with  attention for supertile s-1
  (attention for supertile 3 is folded into stage 3's zipper)

Attention details:
  - qT/kT kept transposed [head_dim, T], 2 heads packed per 128 partitions;
    scores are computed transposed (sT[k, q] = k @ qT) with the two heads
    row-packed on the PE array via tile_position, both written into ONE
    [128, 1024] PSUM tile per k-block so the pair issues back-to-back and
    streams concurrently through separate PE row-quadrants.
  - one exp per k-block on ScalarE over the full [128, 1024] tile (dead
    columns hold stale-but-finite PSUM values and are never read downstream);
    causality = column-range restriction + one triangular-mask multiply per
    diagonal 128-block (on GpSimd, which is otherwise idle).
  - v is kept natural [T, 64] per head with a ones column appended, so the
    AV matmul emits softmax denominators as row 64 of the accumulator.
  - normalization: reciprocal_approx_fast on the denominator row, broadcast
    across partitions with gpsimd.partition_broadcast, then one multiply.
    (The baseline's fp32 rank-1 broadcast matmuls + bit-exact reciprocal
    cost 31us of PE + 53us of DVE; this costs ~1us of each.)
  - proj consumes yT directly; per-core output is a partial [T, C] product
    summed on the host (row-parallel all-reduce).
"""

import sys

for _p in ("/opt/trn_rl_repo",):
    if _p not in sys.path:
        sys.path.insert(0, _p)

import ml_dtypes
import numpy as np

import concourse.bacc as bacc
import concourse.mybir as mybir
import concourse.tile as tile
from concourse.alu_op_type import AluOpType
from concourse.bass_utils import run_bass_kernel_spmd

F32 = mybir.dt.float32
BF16 = mybir.dt.bfloat16
NPBF = ml_dtypes.bfloat16
EXP = mybir.ActivationFunctionType.Exp

B, T, C = 2, 2048, 1024
H, HD = 16, 64
HPC = 4          # heads per core
NPAIR = 2        # head pairs per core
CL = HPC * HD    # 256 local channels
NCORES = 8
SCALE = 0.125    # 1/sqrt(64), folded into wq host-side

TT5 = T // 512   # 4  t supertiles / q supertiles / pipeline stages
TT1 = T // 128   # 16 t tiles / k blocks
CCH = C // 128   # 8  contraction chunks


def _build_program():
    nc = bacc.Bacc("TRN2", target_bir_lowering=False, debug=False)

    xT_d = nc.dram_tensor("xT", [C, T], BF16, kind="ExternalInput").ap()
    wq_d = nc.dram_tensor("wq", [C, CL], BF16, kind="ExternalInput").ap()
    wk_d = nc.dram_tensor("wk", [C, CL], BF16, kind="ExternalInput").ap()
    wv_d = nc.dram_tensor("wv", [C, CL], BF16, kind="ExternalInput").ap()
    wp_d = nc.dram_tensor("wp", [CL, C], BF16, kind="ExternalInput").ap()
    bqs_d = nc.dram_tensor("bqs", [128, NPAIR], F32, kind="ExternalInput").ap()
    bks_d = nc.dram_tensor("bks", [128, NPAIR], F32, kind="ExternalInput").ap()
    bvr_d = nc.dram_tensor("bvr", [1, 2 * CL], BF16, kind="ExternalInput").ap()
    ones1_d = nc.dram_tensor("ones1", [1, 128], BF16, kind="ExternalInput").ap()
    mtri_d = nc.dram_tensor("mtri", [128, 128], BF16, kind="ExternalInput").ap()
    yp_d = nc.dram_tensor("yp", [T, C], F32, kind="ExternalOutput").ap()

    with tile.TileContext(nc) as tc:
        _attn_kernel(tc, xT_d, wq_d, wk_d, wv_d, wp_d, bqs_d, bks_d, bvr_d,
                     ones1_d, mtri_d, yp_d)
    nc.compile()
    return nc


def _attn_kernel(tc, xT_d, wq_d, wk_d, wv_d, wp_d, bqs_d, bks_d, bvr_d,
                 ones1_d, mtri_d, yp_d):
    nc = tc.nc
    mm = nc.tensor.matmul

    with (
        tc.tile_pool(name="const", bufs=1) as cpool,
        tc.tile_pool(name="big", bufs=1) as bigpool,
        tc.tile_pool(name="work", bufs=3) as wkpool,
        tc.tile_pool(name="outp", bufs=3) as opool,
        tc.tile_pool(name="spool", bufs=2, space="PSUM") as spool,
        tc.tile_pool(name="ytpool", bufs=1, space="PSUM") as ytpool,
        tc.tile_pool(name="rot", bufs=2, space="PSUM") as rotpool,
    ):
        # ---- constants ----
        bqs = cpool.tile([128, NPAIR], F32)
        nc.sync.dma_start(bqs, bqs_d)
        bks = cpool.tile([128, NPAIR], F32)
        nc.sync.dma_start(bks, bks_d)
        bvr2 = cpool.tile([1, 2 * CL], BF16)
        nc.sync.dma_start(bvr2, bvr_d)
        ones1 = cpool.tile([1, 128], BF16)
        nc.sync.dma_start(ones1, ones1_d)
        mtri = cpool.tile([128, 128], BF16)
        nc.sync.dma_start(mtri, mtri_d)

        # ---- SBUF residents ----
        xt = bigpool.tile([128, CCH, T], BF16)          # x^T chunks
        wqt = bigpool.tile([128, CCH, CL], BF16)
        wkt = bigpool.tile([128, CCH, CL], BF16)
        wvt = bigpool.tile([128, CCH, CL], BF16)
        wpt = bigpool.tile([128, NPAIR, C], BF16)       # proj weight chunks
        qt = bigpool.tile([128, NPAIR, T], BF16)        # q^T (scaled+biased)
        kt = bigpool.tile([128, NPAIR, T], BF16)        # k^T (biased)
        vt = bigpool.tile([128, TT1, HPC, HD + 1], BF16)  # v natural + ones
        yt = bigpool.tile([128, NPAIR, T], BF16)        # attn out ^T (normed)

        # input DMA: supertile-0 slices of x interleaved with the qkv
        # weights (everything stage 0 touches lands first), then the rest.
        for c in range(CCH):
            nc.sync.dma_start(xt[:, c, 0:512], xT_d[c * 128:(c + 1) * 128, 0:512])
            for w_sb, w_dr in ((wqt, wq_d), (wkt, wk_d), (wvt, wv_d)):
                nc.sync.dma_start(w_sb[:, c, :], w_dr[c * 128:(c + 1) * 128, :])
        for st in range(1, TT5):
            for c in range(CCH):
                nc.sync.dma_start(xt[:, c, st * 512:(st + 1) * 512],
                                  xT_d[c * 128:(c + 1) * 128,
                                       st * 512:(st + 1) * 512])
            if st == 1:
                for p in range(NPAIR):
                    nc.sync.dma_start(wpt[:, p, :],
                                      wp_d[p * 128:(p + 1) * 128, :])

        for tt in range(TT1):
            nc.vector.memset(vt[:, tt, :, HD:HD + 1], 1.0)

        # ================= step builders =================

        def qk_tile_step(st, p, w_sb, dst, bias):
            def run():
                pst = rotpool.tile([128, 512], F32, tag="rot",
                                   name=f"pqk_{st}_{p}_{id(w_sb)}")
                for c in range(CCH):
                    mm(pst,
                       w_sb[:, c, p * 128:(p + 1) * 128],
                       xt[:, c, st * 512:(st + 1) * 512],
                       start=(c == 0), stop=(c == CCH - 1))
                nc.vector.tensor_scalar_add(
                    dst[:, p, st * 512:(st + 1) * 512], pst, bias[:, p:p + 1])
            return run

        def v_tile_step(st, half):
            tt0 = 4 * st + 2 * half
            def run():
                psv = rotpool.tile([128, 512], F32, tag="rot",
                                   name=f"pv_{tt0}")
                # bias first: ONE start=True covering the tile (PSUM start
                # resets the whole 2KB bank, so per-half starts would clobber
                # each other's accumulation)
                mm(psv, ones1, bvr2, start=True, stop=False)
                for c in range(CCH):
                    for j in range(2):
                        mm(psv[:, 256 * j:256 * (j + 1)],
                           xt[:, c, (tt0 + j) * 128:(tt0 + j + 1) * 128],
                           wvt[:, c, :],
                           start=False,
                           stop=(c == CCH - 1 and j == 1))
                for j in range(2):
                    nc.vector.tensor_copy(
                        vt[:, tt0 + j, 0:HPC, 0:HD],
                        psv[:, 256 * j:256 * (j + 1)])
            return run

        def qkv_steps(st):
            steps = []
            for p in range(NPAIR):
                steps.append(qk_tile_step(st, p, wqt, qt, bqs))
                steps.append(qk_tile_step(st, p, wkt, kt, bks))
            steps.append(v_tile_step(st, 0))
            steps.append(v_tile_step(st, 1))
            return steps

        def att_steps(qst):
            q0 = qst * 512
            nkb = 4 * qst + 4
            steps = []
            ytps = {}
            ex_tiles = {}

            def score_step(p, kb):
                j = kb - 4 * qst
                vlo = 128 * j if j >= 0 else 0
                def run():
                    stile = spool.tile([128, 1024], F32, tag="s",
                                       name=f"s_{qst}_{p}_{kb}")
                    for hs in range(2):
                        r = slice(64 * hs, 64 * hs + 64)
                        mm(stile[:, 512 * hs + vlo:512 * (hs + 1)],
                           kt[r, p, kb * 128:(kb + 1) * 128],
                           qt[r, p, q0 + vlo:q0 + 512],
                           tile_position=(64 * hs, 0),
                           start=True, stop=True)
                    ex = wkpool.tile([128, 1024], BF16, tag="ex",
                                     name=f"ex_{qst}_{p}_{kb}", bufs=4)
                    nc.scalar.activation(ex, stile, EXP)
                    if j >= 0:
                        for hs in range(2):
                            band = slice(512 * hs + vlo, 512 * hs + vlo + 128)
                            nc.vector.tensor_mul(ex[:, band], ex[:, band], mtri)
                    ex_tiles[(p, kb)] = ex
                return run

            def av_step(p, kb):
                j = kb - 4 * qst
                vlo = 128 * j if j >= 0 else 0
                def run():
                    ex = ex_tiles[(p, kb)]
                    for hs in range(2):
                        mm(ytps[p][:, 512 * hs + vlo:512 * (hs + 1)],
                           vt[:, kb, 2 * p + hs, :],
                           ex[:, 512 * hs + vlo:512 * (hs + 1)],
                           start=(kb == 0), stop=(kb == nkb - 1))
                return run

            def alloc_step(p):
                def run():
                    # both hs halves in one tile, split at the 512-column
                    # (= PSUM bank) boundary so each hs's start=True resets
                    # only its own bank
                    ytps[p] = ytpool.tile([HD + 1, 1024], F32, tag="yt",
                                          name=f"ytps_{qst}_{p}")
                return run

            def norm_step(p):
                def run():
                    # standard copy first: it carries the dependency on the
                    # AV matmuls (the custom-DVE reciprocal read the PSUM
                    # accumulator before AV finished when reading directly),
                    # and the in-order DVE queue then orders the custom op.
                    dsum = wkpool.tile([1, 1024], F32, tag="dsum",
                                       name=f"dsum_{qst}_{p}")
                    nc.vector.tensor_copy(dsum, ytps[p][HD:HD + 1, :])
                    sinv = wkpool.tile([1, 1024], F32, tag="sinv",
                                       name=f"sinv_{qst}_{p}")
                    nc.vector.reciprocal_approx_fast(sinv, dsum)
                    # broadcast 1/denom across partitions with a pair of
                    # bf16 rank-1 matmuls in opposite PE column-quadrants
                    # (GpSimd partition_broadcast thrashes the gpsimd
                    # library and costs ~6us in reloads per use)
                    sinv_bf = wkpool.tile([1, 1024], BF16, tag="sinvb",
                                          name=f"sinvb_{qst}_{p}")
                    nc.vector.tensor_copy(sinv_bf, sinv)
                    rb = rotpool.tile([128, 512], F32, tag="rot",
                                      name=f"rb_{qst}_{p}")
                    for hs in range(2):
                        mm(rb[64 * hs:64 * hs + 64, :],
                           ones1[:, 0:64],
                           sinv_bf[:, 512 * hs:512 * (hs + 1)],
                           tile_position=(0, 64 * hs),
                           start=True, stop=True)
                    rbs = wkpool.tile([128, 512], F32, tag="rbs",
                                      name=f"rbs_{qst}_{p}")
                    nc.vector.tensor_copy(rbs, rb)
                    for hs in range(2):
                        nc.vector.tensor_mul(
                            yt[64 * hs:64 * hs + 64, p, q0:q0 + 512],
                            ytps[p][0:HD, 512 * hs:512 * (hs + 1)],
                            rbs[64 * hs:64 * hs + 64, :])
                return run

            def proj_step(tt, nh):
                def run():
                    pso = rotpool.tile([128, 512], F32, tag="rot",
                                       name=f"pso_{tt}_{nh}")
                    for p in range(NPAIR):
                        mm(pso,
                           yt[:, p, tt * 128:(tt + 1) * 128],
                           wpt[:, p, nh * 512:(nh + 1) * 512],
                           start=(p == 0), stop=(p == NPAIR - 1))
                    osb = opool.tile([128, 512], F32, tag="osb",
                                     name=f"osb_{tt}_{nh}")
                    nc.vector.tensor_copy(osb, pso)
                    nc.sync.dma_start(
                        yp_d[tt * 128:(tt + 1) * 128,
                             nh * 512:(nh + 1) * 512], osb)
                return run

            for p in range(NPAIR):
                steps.append(alloc_step(p))
                # AV lags scores by 2 k-blocks so the exp (ScalarE) and its
                # semaphore hops hide behind two score+filler slots
                for kb in range(nkb):
                    steps.append(score_step(p, kb))
                    if kb >= 2:
                        steps.append(av_step(p, kb - 2))
                steps.append(av_step(p, nkb - 2))
                steps.append(av_step(p, nkb - 1))
                steps.append(norm_step(p))
            for tt in range(4 * qst, 4 * qst + 4):
                for nh in range(2):
                    steps.append(proj_step(tt, nh))
            return steps

        def zipper(a, b, b_end=None):
            """Interleave two step lists; b is paced so it finishes by
            a-index b_end (default: end of a)."""
            na, nb = len(a), len(b)
            if b_end is None:
                b_end = na
            ia = ib = 0
            while ia < na or ib < nb:
                if ib >= nb or (ia < na and ia * nb <= ib * b_end):
                    a[ia]()
                    ia += 1
                else:
                    b[ib]()
                    ib += 1

        # ================= pipeline =================
        for s in qkv_steps(0):
            s()
        zipper(att_steps(0), qkv_steps(1))
        zipper(att_steps(1), qkv_steps(2))
        a2, a3 = att_steps(2), att_steps(3)
        # qkv(3) must be fully emitted before att(3) reads its q/k/v
        zipper(a2 + a3, qkv_steps(3), b_end=len(a2))


_CACHE = {}


def _get_nc():
    if "nc" not in _CACHE:
        _CACHE["nc"] = _build_program()
    return _CACHE["nc"]


def make_in_maps(x, w_attn, b_attn):
    """Shard the full inputs into 8 per-core input maps."""
    x = np.asarray(x, dtype=np.float32)
    w_attn = np.asarray(w_attn, dtype=np.float32)
    b_attn = np.asarray(b_attn, dtype=np.float32)
    ones1 = np.ones((1, 128), dtype=NPBF)
    mtri = (np.arange(128)[None, :] >= np.arange(128)[:, None]).astype(NPBF)
    in_maps = []
    for core in range(NCORES):
        b, g = divmod(core, 4)
        cs = slice(g * CL, (g + 1) * CL)
        ks = slice(C + g * CL, C + (g + 1) * CL)
        vs = slice(2 * C + g * CL, 2 * C + (g + 1) * CL)
        bv = b_attn[vs].reshape(1, CL).astype(NPBF)
        in_maps.append({
            "xT": np.ascontiguousarray(x[b].T).astype(NPBF),
            "wq": np.ascontiguousarray(SCALE * w_attn[:, cs]).astype(NPBF),
            "wk": np.ascontiguousarray(w_attn[:, ks]).astype(NPBF),
            "wv": np.ascontiguousarray(w_attn[:, vs]).astype(NPBF),
            "wp": None,  # filled by caller (needs w_proj)
            "bqs": np.ascontiguousarray(
                (SCALE * b_attn[cs]).reshape(NPAIR, 128).T),
            "bks": np.ascontiguousarray(b_attn[ks].reshape(NPAIR, 128).T),
            "bvr": np.concatenate([bv, bv], axis=1),
            "ones1": ones1,
            "mtri": mtri,
        })
    return in_maps


def kernel(x, w_attn, b_attn, w_proj, b_proj, _trace=False):
    w_proj = np.asarray(w_proj, dtype=np.float32)
    b_proj = np.asarray(b_proj, dtype=np.float32)
    in_maps = make_in_maps(x, w_attn, b_attn)
    for core in range(NCORES):
        g = core % 4
        in_maps[core]["wp"] = np.ascontiguousarray(
            w_proj[g * CL:(g + 1) * CL, :]).astype(NPBF)
    nc = _get_nc()
    res = run_bass_kernel_spmd(nc, in_maps, core_ids=list(range(NCORES)),
                               trace=_trace)
    out = np.zeros((B, T, C), dtype=np.float32)
    for core in range(NCORES):
        out[core // 4] += res.results[core]["yp"]
    out += b_proj
    if _trace:
        kernel.last_result = res
    return out


# revision 21
# speedup vs baseline: 1.4285x; 1.0762x over previous
"""Causal self-attention Trainium2 kernel (B=2, T=2048, C=1024, H=16).

Sharding: 8 cores = 2 batches x 4 head-groups (4 heads/core, Megatron-style
column-parallel QKV + row-parallel proj; the row-parallel all-reduce is the
host-side partial sum in `kernel`).

Per-core schedule (bf16 matmul operands, fp32 PSUM accumulation), software-
pipelined so the in-order Tensor queue never drains:

  stage 0:  QKV for t-supertile 0
  stage s:  QKV for supertile s  zippered with  attention for supertile s-1
  (attention for supertile 3 is folded into stage 3's zipper)

Attention details:
  - qT/kT kept transposed [head_dim, T], 2 heads packed per 128 partitions;
    scores are computed transposed (sT[k, q] = k @ qT) with the two heads
    row-packed on the PE array via tile_position, both written into ONE
    [128, 1024] PSUM tile per k-block so the pair issues back-to-back and
    streams concurrently through separate PE row-quadrants.
  - one exp per k-block on ScalarE over the full [128, 1024] tile (dead
    columns hold stale-but-finite PSUM values and are never read downstream);
    causality = column-range restriction + one triangular-mask multiply per
    diagonal 128-block (on GpSimd, which is otherwise idle).
  - v is kept natural [T, 64] per head with a ones column appended, so the
    AV matmul emits softmax denominators as row 64 of the accumulator.
  - normalization: reciprocal_approx_fast on the denominator row, broadcast
    across partitions with gpsimd.partition_broadcast, then one multiply.
    (The baseline's fp32 rank-1 broadcast matmuls + bit-exact reciprocal
    cost 31us of PE + 53us of DVE; this costs ~1us of each.)
  - proj consumes yT directly; per-core output is a partial [T, C] product
    summed on the host (row-parallel all-reduce).
"""

import sys

for _p in ("/opt/trn_rl_repo",):
    if _p not in sys.path:
        sys.path.insert(0, _p)

import ml_dtypes
import numpy as np

import concourse.bacc as bacc
import concourse.mybir as mybir
import concourse.tile as tile
from concourse.alu_op_type import AluOpType
from concourse.bass_utils import run_bass_kernel_spmd

F32 = mybir.dt.float32
BF16 = mybir.dt.bfloat16
NPBF = ml_dtypes.bfloat16
EXP = mybir.ActivationFunctionType.Exp

B, T, C = 2, 2048, 1024
H, HD = 16, 64
HPC = 4          # heads per core
NPAIR = 2        # head pairs per core
CL = HPC * HD    # 256 local channels
NCORES = 8
SCALE = 0.125    # 1/sqrt(64), folded into wq host-side

TT5 = T // 512   # 4  t supertiles / q supertiles / pipeline stages
TT1 = T // 128   # 16 t tiles / k blocks
CCH = C // 128   # 8  contraction chunks


def _build_program():
    nc = bacc.Bacc("TRN2", target_bir_lowering=False, debug=False)

    xT_d = nc.dram_tensor("xT", [C, T], BF16, kind="ExternalInput").ap()
    wq_d = nc.dram_tensor("wq", [C, CL], BF16, kind="ExternalInput").ap()
    wk_d = nc.dram_tensor("wk", [C, CL], BF16, kind="ExternalInput").ap()
    wv_d = nc.dram_tensor("wv", [C, CL], BF16, kind="ExternalInput").ap()
    wp_d = nc.dram_tensor("wp", [CL, C], BF16, kind="ExternalInput").ap()
    bqs_d = nc.dram_tensor("bqs", [128, NPAIR], F32, kind="ExternalInput").ap()
    bks_d = nc.dram_tensor("bks", [128, NPAIR], F32, kind="ExternalInput").ap()
    bvr_d = nc.dram_tensor("bvr", [1, 2 * CL], BF16, kind="ExternalInput").ap()
    ones1_d = nc.dram_tensor("ones1", [1, 128], BF16, kind="ExternalInput").ap()
    mtri_d = nc.dram_tensor("mtri", [128, 128], BF16, kind="ExternalInput").ap()
    yp_d = nc.dram_tensor("yp", [T, C], mybir.dt.float16,
                          kind="ExternalOutput").ap()

    with tile.TileContext(nc) as tc:
        _attn_kernel(tc, xT_d, wq_d, wk_d, wv_d, wp_d, bqs_d, bks_d, bvr_d,
                     ones1_d, mtri_d, yp_d)
    nc.compile()
    return nc


def _attn_kernel(tc, xT_d, wq_d, wk_d, wv_d, wp_d, bqs_d, bks_d, bvr_d,
                 ones1_d, mtri_d, yp_d):
    nc = tc.nc
    mm = nc.tensor.matmul

    with (
        tc.tile_pool(name="const", bufs=1) as cpool,
        tc.tile_pool(name="big", bufs=1) as bigpool,
        tc.tile_pool(name="work", bufs=3) as wkpool,
        tc.tile_pool(name="outp", bufs=3) as opool,
        tc.tile_pool(name="spool", bufs=2, space="PSUM") as spool,
        tc.tile_pool(name="ytpool", bufs=1, space="PSUM") as ytpool,
        tc.tile_pool(name="rot", bufs=2, space="PSUM") as rotpool,
    ):
        # ---- constants ----
        bqs = cpool.tile([128, NPAIR], F32)
        nc.sync.dma_start(bqs, bqs_d)
        bks = cpool.tile([128, NPAIR], F32)
        nc.sync.dma_start(bks, bks_d)
        bvr2 = cpool.tile([1, 2 * CL], BF16)
        nc.sync.dma_start(bvr2, bvr_d)
        ones1 = cpool.tile([1, 128], BF16)
        nc.sync.dma_start(ones1, ones1_d)
        mtri = cpool.tile([128, 128], BF16)
        nc.sync.dma_start(mtri, mtri_d)

        # ---- SBUF residents ----
        xt = bigpool.tile([128, CCH, T], BF16)          # x^T chunks
        wqt = bigpool.tile([128, CCH, CL], BF16)
        wkt = bigpool.tile([128, CCH, CL], BF16)
        wvt = bigpool.tile([128, CCH, CL], BF16)
        wpt = bigpool.tile([128, NPAIR, C], BF16)       # proj weight chunks
        qt = bigpool.tile([128, NPAIR, T], BF16)        # q^T (scaled+biased)
        kt = bigpool.tile([128, NPAIR, T], BF16)        # k^T (biased)
        # v natural [T, 64] per head + a 64-wide ones block: the AV matmul
        # then emits the softmax denominator replicated across PSUM
        # partitions 64..127 for free (ap_size is set by the moving operand),
        # so normalization needs no partition-broadcast at all.
        vt = bigpool.tile([128, TT1, HPC, 2 * HD], BF16)
        yt = bigpool.tile([128, NPAIR, T], BF16)        # attn out ^T (normed)

        # input DMA, batched: one multi-dim DMA per tensor/supertile so the
        # sync queue isn't spending ~400ns of issue cost x 57 descriptors
        # (that serialized stage 0 behind DMA arrival in earlier versions).
        xT_r = xT_d.rearrange("(c p) t -> p c t", p=128)   # [128, CCH, T]
        nc.sync.dma_start(xt[:, :, 0:512], xT_r[:, :, 0:512])
        for w_sb, w_dr in ((wqt, wq_d), (wkt, wk_d), (wvt, wv_d)):
            nc.sync.dma_start(w_sb, w_dr.rearrange("(c p) m -> p c m", p=128))
        for st in range(1, TT5):
            nc.sync.dma_start(xt[:, :, st * 512:(st + 1) * 512],
                              xT_r[:, :, st * 512:(st + 1) * 512])
            if st == 1:
                nc.sync.dma_start(
                    wpt, wp_d.rearrange("(q p) m -> p q m", p=128))

        for tt in range(TT1):
            nc.vector.memset(vt[:, tt, :, HD:HD + 1], 1.0)

        # ================= step builders =================

        def qk_tile_step(st, p, w_sb, dst, bias):
            def run():
                pst = rotpool.tile([128, 512], F32, tag="rot",
                                   name=f"pqk_{st}_{p}_{id(w_sb)}")
                for c in range(CCH):
                    mm(pst,
                       w_sb[:, c, p * 128:(p + 1) * 128],
                       xt[:, c, st * 512:(st + 1) * 512],
                       start=(c == 0), stop=(c == CCH - 1))
                nc.vector.tensor_scalar_add(
                    dst[:, p, st * 512:(st + 1) * 512], pst, bias[:, p:p + 1])
            return run

        def v_tile_step(st, half):
            tt0 = 4 * st + 2 * half
            def run():
                psv = rotpool.tile([128, 512], F32, tag="rot",
                                   name=f"pv_{tt0}")
                # bias first: ONE start=True covering the tile (PSUM start
                # resets the whole 2KB bank, so per-half starts would clobber
                # each other's accumulation)
                mm(psv, ones1, bvr2, start=True, stop=False)
                for c in range(CCH):
                    for j in range(2):
                        mm(psv[:, 256 * j:256 * (j + 1)],
                           xt[:, c, (tt0 + j) * 128:(tt0 + j + 1) * 128],
                           wvt[:, c, :],
                           start=False,
                           stop=(c == CCH - 1 and j == 1))
                for j in range(2):
                    nc.vector.tensor_copy(
                        vt[:, tt0 + j, 0:HPC, 0:HD],
                        psv[:, 256 * j:256 * (j + 1)])
            return run

        def qkv_steps(st):
            steps = []
            for p in range(NPAIR):
                steps.append(qk_tile_step(st, p, wqt, qt, bqs))
                steps.append(qk_tile_step(st, p, wkt, kt, bks))
            steps.append(v_tile_step(st, 0))
            steps.append(v_tile_step(st, 1))
            return steps

        def att_steps(qst):
            q0 = qst * 512
            nkb = 4 * qst + 4
            steps = []
            ytps = {}
            ex_tiles = {}

            def score_step(p, kb):
                j = kb - 4 * qst
                vlo = 128 * j if j >= 0 else 0
                def run():
                    stile = spool.tile([128, 1024], F32, tag="s",
                                       name=f"s_{qst}_{p}_{kb}")
                    for hs in range(2):
                        r = slice(64 * hs, 64 * hs + 64)
                        mm(stile[:, 512 * hs + vlo:512 * (hs + 1)],
                           kt[r, p, kb * 128:(kb + 1) * 128],
                           qt[r, p, q0 + vlo:q0 + 512],
                           tile_position=(64 * hs, 0),
                           start=True, stop=True)
                    ex = wkpool.tile([128, 1024], BF16, tag="ex",
                                     name=f"ex_{qst}_{p}_{kb}", bufs=4)
                    nc.scalar.activation(ex, stile, EXP)
                    if j >= 0:
                        for hs in range(2):
                            band = slice(512 * hs + vlo, 512 * hs + vlo + 128)
                            nc.vector.tensor_mul(ex[:, band], ex[:, band], mtri)
                    ex_tiles[(p, kb)] = ex
                return run

            def av_step(p, kb):
                j = kb - 4 * qst
                vlo = 128 * j if j >= 0 else 0
                def run():
                    ex = ex_tiles[(p, kb)]
                    for hs in range(2):
                        mm(ytps[p][:, 512 * hs + vlo:512 * (hs + 1)],
                           vt[:, kb, 2 * p + hs, :],
                           ex[:, 512 * hs + vlo:512 * (hs + 1)],
                           start=(kb == 0), stop=(kb == nkb - 1))
                return run

            def alloc_step(p):
                def run():
                    # both hs halves in one tile, split at the 512-column
                    # (= PSUM bank) boundary so each hs's start=True resets
                    # only its own bank
                    ytps[p] = ytpool.tile([HD + 1, 1024], F32, tag="yt",
                                          name=f"ytps_{qst}_{p}")
                return run

            def norm_step(p):
                def run():
                    # standard copy first: it carries the dependency on the
                    # AV matmuls (the custom-DVE reciprocal read the PSUM
                    # accumulator before AV finished when reading directly),
                    # and the in-order DVE queue then orders the custom op.
                    dsum = wkpool.tile([1, 1024], F32, tag="dsum",
                                       name=f"dsum_{qst}_{p}")
                    nc.vector.tensor_copy(dsum, ytps[p][HD:HD + 1, :])
                    sinv = wkpool.tile([1, 1024], F32, tag="sinv",
                                       name=f"sinv_{qst}_{p}")
                    nc.vector.reciprocal_approx_fast(sinv, dsum)
                    # broadcast 1/denom across partitions with a pair of
                    # bf16 rank-1 matmuls in opposite PE column-quadrants
                    # (GpSimd partition_broadcast thrashes the gpsimd
                    # library and costs ~6us in reloads per use)
                    sinv_bf = wkpool.tile([1, 1024], BF16, tag="sinvb",
                                          name=f"sinvb_{qst}_{p}")
                    nc.vector.tensor_copy(sinv_bf, sinv)
                    rb = rotpool.tile([128, 512], F32, tag="rot",
                                      name=f"rb_{qst}_{p}")
                    for hs in range(2):
                        mm(rb[64 * hs:64 * hs + 64, :],
                           ones1[:, 0:64],
                           sinv_bf[:, 512 * hs:512 * (hs + 1)],
                           tile_position=(0, 64 * hs),
                           start=True, stop=True)
                    rbs = wkpool.tile([128, 512], F32, tag="rbs",
                                      name=f"rbs_{qst}_{p}")
                    nc.vector.tensor_copy(rbs, rb)
                    for hs in range(2):
                        nc.vector.tensor_mul(
                            yt[64 * hs:64 * hs + 64, p, q0:q0 + 512],
                            ytps[p][0:HD, 512 * hs:512 * (hs + 1)],
                            rbs[64 * hs:64 * hs + 64, :])
                return run

            def proj_step(tt, nh):
                def run():
                    pso = rotpool.tile([128, 512], F32, tag="rot",
                                       name=f"pso_{tt}_{nh}")
                    for p in range(NPAIR):
                        mm(pso,
                           yt[:, p, tt * 128:(tt + 1) * 128],
                           wpt[:, p, nh * 512:(nh + 1) * 512],
                           start=(p == 0), stop=(p == NPAIR - 1))
                    osb = opool.tile([128, 512], mybir.dt.float16,
                                     tag="osb", name=f"osb_{tt}_{nh}")
                    nc.vector.tensor_copy(osb, pso)
                    nc.sync.dma_start(
                        yp_d[tt * 128:(tt + 1) * 128,
                             nh * 512:(nh + 1) * 512], osb)
                return run

            for p in range(NPAIR):
                steps.append(alloc_step(p))
                # AV lags scores by 2 k-blocks so the exp (ScalarE) and its
                # semaphore hops hide behind two score+filler slots
                for kb in range(nkb):
                    steps.append(score_step(p, kb))
                    if kb >= 2:
                        steps.append(av_step(p, kb - 2))
                steps.append(av_step(p, nkb - 2))
                steps.append(av_step(p, nkb - 1))
                steps.append(norm_step(p))
            for tt in range(4 * qst, 4 * qst + 4):
                for nh in range(2):
                    steps.append(proj_step(tt, nh))
            return steps

        def zipper(a, b, b_end=None):
            """Interleave two step lists; b is paced so it finishes by
            a-index b_end (default: end of a)."""
            na, nb = len(a), len(b)
            if b_end is None:
                b_end = na
            ia = ib = 0
            while ia < na or ib < nb:
                if ib >= nb or (ia < na and ia * nb <= ib * b_end):
                    a[ia]()
                    ia += 1
                else:
                    b[ib]()
                    ib += 1

        # ================= pipeline =================
        for s in qkv_steps(0):
            s()
        zipper(att_steps(0), qkv_steps(1))
        zipper(att_steps(1), qkv_steps(2))
        a2, a3 = att_steps(2), att_steps(3)
        # qkv(3) must be fully emitted before att(3) reads its q/k/v
        zipper(a2 + a3, qkv_steps(3), b_end=len(a2))


_CACHE = {}


def _get_nc():
    if "nc" not in _CACHE:
        _CACHE["nc"] = _build_program()
    return _CACHE["nc"]


def make_in_maps(x, w_attn, b_attn):
    """Shard the full inputs into 8 per-core input maps."""
    x = np.asarray(x, dtype=np.float32)
    w_attn = np.asarray(w_attn, dtype=np.float32)
    b_attn = np.asarray(b_attn, dtype=np.float32)
    ones1 = np.ones((1, 128), dtype=NPBF)
    mtri = (np.arange(128)[None, :] >= np.arange(128)[:, None]).astype(NPBF)
    in_maps = []
    for core in range(NCORES):
        b, g = divmod(core, 4)
        cs = slice(g * CL, (g + 1) * CL)
        ks = slice(C + g * CL, C + (g + 1) * CL)
        vs = slice(2 * C + g * CL, 2 * C + (g + 1) * CL)
        bv = b_attn[vs].reshape(1, CL).astype(NPBF)
        in_maps.append({
            "xT": np.ascontiguousarray(x[b].T).astype(NPBF),
            "wq": np.ascontiguousarray(SCALE * w_attn[:, cs]).astype(NPBF),
            "wk": np.ascontiguousarray(w_attn[:, ks]).astype(NPBF),
            "wv": np.ascontiguousarray(w_attn[:, vs]).astype(NPBF),
            "wp": None,  # filled by caller (needs w_proj)
            "bqs": np.ascontiguousarray(
                (SCALE * b_attn[cs]).reshape(NPAIR, 128).T),
            "bks": np.ascontiguousarray(b_attn[ks].reshape(NPAIR, 128).T),
            "bvr": np.concatenate([bv, bv], axis=1),
            "ones1": ones1,
            "mtri": mtri,
        })
    return in_maps


def kernel(x, w_attn, b_attn, w_proj, b_proj, _trace=False):
    w_proj = np.asarray(w_proj, dtype=np.float32)
    b_proj = np.asarray(b_proj, dtype=np.float32)
    in_maps = make_in_maps(x, w_attn, b_attn)
    for core in range(NCORES):
        g = core % 4
        in_maps[core]["wp"] = np.ascontiguousarray(
            w_proj[g * CL:(g + 1) * CL, :]).astype(NPBF)
    nc = _get_nc()
    res = run_bass_kernel_spmd(nc, in_maps, core_ids=list(range(NCORES)),
                               trace=_trace)
    out = np.zeros((B, T, C), dtype=np.float32)
    for core in range(NCORES):
        out[core // 4] += res.results[core]["yp"]
    out += b_proj
    if _trace:
        kernel.last_result = res
    return out


# revision 25
# speedup vs baseline: 1.5915x; 1.1141x over previous
"""Causal self-attention Trainium2 kernel (B=2, T=2048, C=1024, H=16).

Sharding: 8 cores = 2 batches x 4 head-groups (4 heads/core, Megatron-style
column-parallel QKV + row-parallel proj; the row-parallel all-reduce is the
host-side partial sum in `kernel`).

Per-core schedule (bf16 matmul operands, fp32 PSUM accumulation), software-
pipelined so the in-order Tensor queue never drains:

  stage 0:  QKV for t-supertile 0
  stage s:  QKV for supertile s  zippered with  attention for supertile s-1
  (attention for supertile 3 is folded into stage 3's zipper)

Attention details:
  - qT/kT kept transposed [head_dim, T], 2 heads packed per 128 partitions;
    scores are computed transposed (sT[k, q] = k @ qT) with the two heads
    row-packed on the PE array via tile_position, both written into ONE
    [128, 1024] PSUM tile per k-block so the pair issues back-to-back and
    streams concurrently through separate PE row-quadrants.
  - one exp per k-block on ScalarE over the full [128, 1024] tile (dead
    columns hold stale-but-finite PSUM values and are never read downstream);
    causality = column-range restriction + one triangular-mask multiply per
    diagonal 128-block (on GpSimd, which is otherwise idle).
  - v is kept natural [T, 64] per head with a ones column appended, so the
    AV matmul emits softmax denominators as row 64 of the accumulator.
  - normalization: reciprocal_approx_fast on the denominator row, broadcast
    across partitions with gpsimd.partition_broadcast, then one multiply.
    (The baseline's fp32 rank-1 broadcast matmuls + bit-exact reciprocal
    cost 31us of PE + 53us of DVE; this costs ~1us of each.)
  - proj consumes yT directly; per-core output is a partial [T, C] product
    summed on the host (row-parallel all-reduce).
"""

import sys

for _p in ("/opt/trn_rl_repo",):
    if _p not in sys.path:
        sys.path.insert(0, _p)

import ml_dtypes
import numpy as np

import concourse.bacc as bacc
import concourse.mybir as mybir
import concourse.tile as tile
from concourse.alu_op_type import AluOpType
from concourse.bass_utils import run_bass_kernel_spmd

F32 = mybir.dt.float32
BF16 = mybir.dt.bfloat16
NPBF = ml_dtypes.bfloat16
EXP = mybir.ActivationFunctionType.Exp

B, T, C = 2, 2048, 1024
H, HD = 16, 64
HPC = 4          # heads per core
NPAIR = 2        # head pairs per core
CL = HPC * HD    # 256 local channels
NCORES = 8
SCALE = 0.125    # 1/sqrt(64), folded into wq host-side

TT5 = T // 512   # 4  t supertiles / q supertiles / pipeline stages
TT1 = T // 128   # 16 t tiles / k blocks
CCH = C // 128   # 8  contraction chunks


def _build_program():
    nc = bacc.Bacc("TRN2", target_bir_lowering=False, debug=False)

    xT_d = nc.dram_tensor("xT", [C, T], BF16, kind="ExternalInput").ap()
    wq_d = nc.dram_tensor("wq", [C, CL], BF16, kind="ExternalInput").ap()
    wk_d = nc.dram_tensor("wk", [C, CL], BF16, kind="ExternalInput").ap()
    wv_d = nc.dram_tensor("wv", [C, CL], BF16, kind="ExternalInput").ap()
    wp_d = nc.dram_tensor("wp", [CL, C], BF16, kind="ExternalInput").ap()
    bqs_d = nc.dram_tensor("bqs", [128, NPAIR], F32, kind="ExternalInput").ap()
    bks_d = nc.dram_tensor("bks", [128, NPAIR], F32, kind="ExternalInput").ap()
    bvr_d = nc.dram_tensor("bvr", [1, 2 * CL], BF16, kind="ExternalInput").ap()
    ones1_d = nc.dram_tensor("ones1", [1, 128], BF16, kind="ExternalInput").ap()
    mtri_d = nc.dram_tensor("mtri", [128, 128], BF16, kind="ExternalInput").ap()
    yp_d = nc.dram_tensor("yp", [T, C], mybir.dt.float16,
                          kind="ExternalOutput").ap()

    with tile.TileContext(nc) as tc:
        _attn_kernel(tc, xT_d, wq_d, wk_d, wv_d, wp_d, bqs_d, bks_d, bvr_d,
                     ones1_d, mtri_d, yp_d)
    nc.compile()
    return nc


def _attn_kernel(tc, xT_d, wq_d, wk_d, wv_d, wp_d, bqs_d, bks_d, bvr_d,
                 ones1_d, mtri_d, yp_d):
    nc = tc.nc
    mm = nc.tensor.matmul

    with (
        tc.tile_pool(name="const", bufs=1) as cpool,
        tc.tile_pool(name="big", bufs=1) as bigpool,
        tc.tile_pool(name="work", bufs=3) as wkpool,
        tc.tile_pool(name="outp", bufs=3) as opool,
        tc.tile_pool(name="spool", bufs=2, space="PSUM") as spool,
        tc.tile_pool(name="ytpool", bufs=1, space="PSUM") as ytpool,
        tc.tile_pool(name="rot", bufs=2, space="PSUM") as rotpool,
    ):
        # ---- constants ----
        bqs = cpool.tile([128, NPAIR], F32)
        nc.sync.dma_start(bqs, bqs_d)
        bks = cpool.tile([128, NPAIR], F32)
        nc.sync.dma_start(bks, bks_d)
        bvr2 = cpool.tile([1, 2 * CL], BF16)
        nc.sync.dma_start(bvr2, bvr_d)
        ones1 = cpool.tile([1, 128], BF16)
        nc.sync.dma_start(ones1, ones1_d)
        mtri = cpool.tile([128, 128], BF16)
        nc.sync.dma_start(mtri, mtri_d)

        # ---- SBUF residents ----
        xt = bigpool.tile([128, CCH, T], BF16)          # x^T chunks
        wqt = bigpool.tile([128, CCH, CL], BF16)
        wkt = bigpool.tile([128, CCH, CL], BF16)
        wvt = bigpool.tile([128, CCH, CL], BF16)
        wpt = bigpool.tile([128, NPAIR, C], BF16)       # proj weight chunks
        qt = bigpool.tile([128, NPAIR, T], BF16)        # q^T (scaled+biased)
        kt = bigpool.tile([128, NPAIR, T], BF16)        # k^T (biased)
        # v natural [T, 64] per head + a 64-wide ones block: the AV matmul
        # then emits the softmax denominator replicated across PSUM
        # partitions 64..127 for free (ap_size is set by the moving operand),
        # so normalization needs no partition-broadcast at all.
        vt = bigpool.tile([128, TT1, HPC, 2 * HD], BF16)
        yt = bigpool.tile([128, NPAIR, T], BF16)        # attn out ^T (normed)

        # input DMA, batched: one multi-dim DMA per tensor/supertile so the
        # sync queue isn't spending ~400ns of issue cost x 57 descriptors
        # (that serialized stage 0 behind DMA arrival in earlier versions).
        xT_r = xT_d.rearrange("(c p) t -> p c t", p=128)   # [128, CCH, T]
        nc.sync.dma_start(xt[:, :, 0:512], xT_r[:, :, 0:512])
        for w_sb, w_dr in ((wqt, wq_d), (wkt, wk_d), (wvt, wv_d)):
            nc.sync.dma_start(w_sb, w_dr.rearrange("(c p) m -> p c m", p=128))
        for st in range(1, TT5):
            nc.sync.dma_start(xt[:, :, st * 512:(st + 1) * 512],
                              xT_r[:, :, st * 512:(st + 1) * 512])
            if st == 1:
                nc.sync.dma_start(
                    wpt, wp_d.rearrange("(q p) m -> p q m", p=128))

        for tt in range(TT1):
            nc.vector.memset(vt[:, tt, :, HD:2 * HD], 1.0)
        # pre-load the gpsimd standard library before the first mask multiply
        gpw = cpool.tile([1, 8], BF16)
        nc.gpsimd.memset(gpw, 0.0)

        # ================= step builders =================

        def qk_tile_step(st, p, w_sb, dst, bias):
            def run():
                pst = rotpool.tile([128, 512], F32, tag="rot",
                                   name=f"pqk_{st}_{p}_{id(w_sb)}")
                for c in range(CCH):
                    mm(pst,
                       w_sb[:, c, p * 128:(p + 1) * 128],
                       xt[:, c, st * 512:(st + 1) * 512],
                       start=(c == 0), stop=(c == CCH - 1))
                nc.vector.tensor_scalar_add(
                    dst[:, p, st * 512:(st + 1) * 512], pst, bias[:, p:p + 1])
            return run

        def v_tile_step(st, half):
            tt0 = 4 * st + 2 * half
            def run():
                psv = rotpool.tile([128, 512], F32, tag="rot",
                                   name=f"pv_{tt0}")
                # bias first: ONE start=True covering the tile (PSUM start
                # resets the whole 2KB bank, so per-half starts would clobber
                # each other's accumulation)
                mm(psv, ones1, bvr2, start=True, stop=False)
                for c in range(CCH):
                    for j in range(2):
                        mm(psv[:, 256 * j:256 * (j + 1)],
                           xt[:, c, (tt0 + j) * 128:(tt0 + j + 1) * 128],
                           wvt[:, c, :],
                           start=False,
                           stop=(c == CCH - 1 and j == 1))
                for j in range(2):
                    nc.vector.tensor_copy(
                        vt[:, tt0 + j, 0:HPC, 0:HD],
                        psv[:, 256 * j:256 * (j + 1)])
            return run

        def qkv_steps(st):
            steps = []
            for p in range(NPAIR):
                steps.append(qk_tile_step(st, p, wqt, qt, bqs))
                steps.append(qk_tile_step(st, p, wkt, kt, bks))
            steps.append(v_tile_step(st, 0))
            steps.append(v_tile_step(st, 1))
            return steps

        def att_steps(qst):
            q0 = qst * 512
            nkb = 4 * qst + 4
            steps = []
            ytps = {}
            ex_tiles = {}

            def score_step(p, kb):
                j = kb - 4 * qst
                vlo = 128 * j if j >= 0 else 0
                def run():
                    stile = spool.tile([128, 1024], F32, tag="s",
                                       name=f"s_{qst}_{p}_{kb}")
                    for hs in range(2):
                        r = slice(64 * hs, 64 * hs + 64)
                        mm(stile[:, 512 * hs + vlo:512 * (hs + 1)],
                           kt[r, p, kb * 128:(kb + 1) * 128],
                           qt[r, p, q0 + vlo:q0 + 512],
                           tile_position=(64 * hs, 0),
                           start=True, stop=True)
                    ex = wkpool.tile([128, 1024], BF16, tag="ex",
                                     name=f"ex_{qst}_{p}_{kb}", bufs=4)
                    nc.scalar.activation(ex, stile, EXP)
                    if j >= 0:
                        for hs in range(2):
                            band = slice(512 * hs + vlo, 512 * hs + vlo + 128)
                            nc.gpsimd.tensor_mul(ex[:, band], ex[:, band], mtri)
                    ex_tiles[(p, kb)] = ex
                return run

            def av_step(p, kb):
                j = kb - 4 * qst
                vlo = 128 * j if j >= 0 else 0
                def run():
                    ex = ex_tiles[(p, kb)]
                    for hs in range(2):
                        mm(ytps[p][:, 512 * hs + vlo:512 * (hs + 1)],
                           vt[:, kb, 2 * p + hs, :],
                           ex[:, 512 * hs + vlo:512 * (hs + 1)],
                           start=(kb == 0), stop=(kb == nkb - 1))
                return run

            def alloc_step(p):
                def run():
                    # both hs halves in one tile, split at the 512-column
                    # (= PSUM bank) boundary so each hs's start=True resets
                    # only its own bank; partitions 0..63 = attn out,
                    # 64..127 = replicated softmax denominators
                    ytps[p] = ytpool.tile([128, 1024], F32, tag="yt",
                                          name=f"ytps_{qst}_{p}")
                return run

            def norm_step(p):
                def run():
                    # standard copy first: it carries the dependency on the
                    # AV matmuls (the custom-DVE reciprocal read the PSUM
                    # accumulator before AV finished when reading directly),
                    # and the in-order DVE queue then orders the custom op.
                    # the copy also shifts the replicated denominators down
                    # to partitions 0..63 (the multiply requires its two
                    # inputs at the same base partition)
                    den = wkpool.tile([64, 1024], F32, tag="den",
                                      name=f"den_{qst}_{p}")
                    nc.vector.tensor_copy(den, ytps[p][64:128, :])
                    sinv = wkpool.tile([64, 1024], F32, tag="sinv",
                                       name=f"sinv_{qst}_{p}")
                    nc.vector.reciprocal_approx_fast(sinv, den)
                    for hs in range(2):
                        nc.vector.tensor_mul(
                            yt[64 * hs:64 * hs + 64, p, q0:q0 + 512],
                            ytps[p][0:HD, 512 * hs:512 * (hs + 1)],
                            sinv[:, 512 * hs:512 * (hs + 1)])
                return run

            def proj_step(tt, nh):
                def run():
                    pso = rotpool.tile([128, 512], F32, tag="rot",
                                       name=f"pso_{tt}_{nh}")
                    for p in range(NPAIR):
                        mm(pso,
                           yt[:, p, tt * 128:(tt + 1) * 128],
                           wpt[:, p, nh * 512:(nh + 1) * 512],
                           start=(p == 0), stop=(p == NPAIR - 1))
                    osb = opool.tile([128, 512], mybir.dt.float16,
                                     tag="osb", name=f"osb_{tt}_{nh}")
                    nc.vector.tensor_copy(osb, pso)
                    nc.sync.dma_start(
                        yp_d[tt * 128:(tt + 1) * 128,
                             nh * 512:(nh + 1) * 512], osb)
                return run

            for p in range(NPAIR):
                steps.append(alloc_step(p))
                # AV lags scores by 2 k-blocks so the exp (ScalarE) and its
                # semaphore hops hide behind two score+filler slots
                for kb in range(nkb):
                    steps.append(score_step(p, kb))
                    if kb >= 2:
                        steps.append(av_step(p, kb - 2))
                steps.append(av_step(p, nkb - 2))
                steps.append(av_step(p, nkb - 1))
                steps.append(norm_step(p))
            for tt in range(4 * qst, 4 * qst + 4):
                for nh in range(2):
                    steps.append(proj_step(tt, nh))
            return steps

        def zipper(a, b, b_end=None):
            """Interleave two step lists; b is paced so it finishes by
            a-index b_end (default: end of a)."""
            na, nb = len(a), len(b)
            if b_end is None:
                b_end = na
            ia = ib = 0
            while ia < na or ib < nb:
                if ib >= nb or (ia < na and ia * nb <= ib * b_end):
                    a[ia]()
                    ia += 1
                else:
                    b[ib]()
                    ib += 1

        # ================= pipeline =================
        for s in qkv_steps(0):
            s()
        zipper(att_steps(0), qkv_steps(1))
        zipper(att_steps(1), qkv_steps(2))
        a2, a3 = att_steps(2), att_steps(3)
        # qkv(3) must be fully emitted before att(3) reads its q/k/v
        zipper(a2 + a3, qkv_steps(3), b_end=len(a2))


_CACHE = {}


def _get_nc():
    if "nc" not in _CACHE:
        _CACHE["nc"] = _build_program()
    return _CACHE["nc"]


def make_in_maps(x, w_attn, b_attn):
    """Shard the full inputs into 8 per-core input maps."""
    x = np.asarray(x, dtype=np.float32)
    w_attn = np.asarray(w_attn, dtype=np.float32)
    b_attn = np.asarray(b_attn, dtype=np.float32)
    ones1 = np.ones((1, 128), dtype=NPBF)
    mtri = (np.arange(128)[None, :] >= np.arange(128)[:, None]).astype(NPBF)
    in_maps = []
    for core in range(NCORES):
        b, g = divmod(core, 4)
        cs = slice(g * CL, (g + 1) * CL)
        ks = slice(C + g * CL, C + (g + 1) * CL)
        vs = slice(2 * C + g * CL, 2 * C + (g + 1) * CL)
        bv = b_attn[vs].reshape(1, CL).astype(NPBF)
        in_maps.append({
            "xT": np.ascontiguousarray(x[b].T).astype(NPBF),
            "wq": np.ascontiguousarray(SCALE * w_attn[:, cs]).astype(NPBF),
            "wk": np.ascontiguousarray(w_attn[:, ks]).astype(NPBF),
            "wv": np.ascontiguousarray(w_attn[:, vs]).astype(NPBF),
            "wp": None,  # filled by caller (needs w_proj)
            "bqs": np.ascontiguousarray(
                (SCALE * b_attn[cs]).reshape(NPAIR, 128).T),
            "bks": np.ascontiguousarray(b_attn[ks].reshape(NPAIR, 128).T),
            "bvr": np.concatenate([bv, bv], axis=1),
            "ones1": ones1,
            "mtri": mtri,
        })
    return in_maps


def kernel(x, w_attn, b_attn, w_proj, b_proj, _trace=False):
    w_proj = np.asarray(w_proj, dtype=np.float32)
    b_proj = np.asarray(b_proj, dtype=np.float32)
    in_maps = make_in_maps(x, w_attn, b_attn)
    for core in range(NCORES):
        g = core % 4
        in_maps[core]["wp"] = np.ascontiguousarray(
            w_proj[g * CL:(g + 1) * CL, :]).astype(NPBF)
    nc = _get_nc()
    res = run_bass_kernel_spmd(nc, in_maps, core_ids=list(range(NCORES)),
                               trace=_trace)
    out = np.zeros((B, T, C), dtype=np.float32)
    for core in range(NCORES):
        out[core // 4] += res.results[core]["yp"]
    out += b_proj
    if _trace:
        kernel.last_result = res
    return out
